# revision 18
# baseline (speedup 1.0000x reference)
"""Trainium2 Bass kernel for CrossFrameSimilarityRefiner.

Computation (per batch element b, fully batch-parallel -> B=8 sharded over 8 cores):
  f = features[:, b]                      # [T, C, P]  T=16, C=256, P=1024
  ss[t,p] = sum_c f^2 ; sm[t,p] = sum_c f ; gm[t,p] = sum_c (f>0)
  S[t,p]  = sm / sqrt(ss)
  scores[t,s] = sum_p S[t,p] * gm[s,p]    # same per-row ranking as reference
  mask diag, top-3 indices, compressed c* = s* - (s* > t)  (faithful bug)
  out[t] = (W/3) @ (f[c*0]+f[c*1]+f[c*2]) + b

Pipeline: startup ~7us -> HBM-bound input stream ~46us (frames 13,14 on the
sync ring first for warmup, 0..12 via software-DGE cast-DMA, frame 15 on the
sync ring LAST so the stats-psum stopper rides a fast completion semaphore
instead of the laggy DGE one) -> bridge (stats evac, PE transposes, score
matmuls, top-k, index math) -> phase C (per t: two DVE gather-adds, 8 PE
matmuls, ACT psum-evac with fused bias, fp16 out-DMA; host casts to fp32).
"""

import numpy as np

import concourse.bacc as bacc
import concourse.bass as bass
import concourse.tile as tile
from concourse import mybir
from concourse.bass_utils import run_bass_kernel_spmd

FP32 = mybir.dt.float32
F16 = mybir.dt.float16
I32 = mybir.dt.int32
U32 = mybir.dt.uint32
AF = mybir.ActivationFunctionType
OP = mybir.AluOpType
ET = mybir.EngineType

N_CORES = 8
BIG = 1.0e30


def _emit(nc, tc, T, C, P, K, handles, debug):
    feat_h = handles["features"]
    out_h = handles["out"]
    CC = C // 128
    PH = P // 512
    PB = P // 128
    DC = C // 128

    with tc.tile_pool(name="persist", bufs=1) as pp:
        wt3_sb = pp.tile([128, CC, C], F16, tag="wt3")
        bcol_sb = pp.tile([128, DC], FP32, tag="bcol")
        esel_sb = pp.tile([128, T * T], F16, tag="esel")
        i16_sb = pp.tile([96, T], FP32, tag="i16")
        diag_sb = pp.tile([T, T], FP32, tag="diag")
        tcolu_sb = pp.tile([T, K], U32, tag="tcolu")

        f16_sb = pp.tile([128, CC, T * P], F16, tag="f16")
        stats_sb = pp.tile([96, P], FP32, tag="stats")
        sm_sb = stats_sb[0:T, :]
        rs_sb = stats_sb[32:32 + T, :]
        gm_sb = stats_sb[64:64 + T, :]
        rst_sb = pp.tile([128, PB, T], FP32, tag="rsT")
        spt_sb = pp.tile([128, PB, T], FP32, tag="SpT")
        mpt_sb = pp.tile([128, PB, T], FP32, tag="MpT")
        scores_sb = pp.tile([T, T], FP32, tag="scores")
        maxv_sb = pp.tile([T, 8], FP32, tag="maxv")
        pad32_sb = pp.tile([32, 32], U32, tag="pad32")
        zt_sb = pp.tile([32, 32], U32, tag="zt")
        gtu_sb = pp.tile([T, K], U32, tag="gtu")

        with tc.tile_pool(name="statsps", bufs=1, space="PSUM") as sps, \
             tc.tile_pool(name="bps", bufs=1, space="PSUM") as bps, \
             tc.tile_pool(name="stream", bufs=4) as sp:
            st_ps = [[sps.tile([96, 512], FP32, tag=f"stp{ph}_{j}",
                               name=f"stp{ph}_{j}") for j in range(3)]
                     for ph in range(PH)]
            trall = bps.tile([128, 3, PB * T], FP32, tag="trall", name="trall")
            sc_ps = bps.tile([T, T], FP32, tag="scps", name="scps")

            nc.gpsimd.memset(pad32_sb[:], 0)

            early = [T - 3, T - 2, T - 1]
            order = [T - 3, T - 2] + list(range(T - 3)) + [T - 1]
            for i, t in enumerate(order):
                f16v = f16_sb[:, :, t * P:(t + 1) * P]
                if t in early:
                    fch = sp.tile([128, CC, P], FP32, tag="fch")
                    for cc in range(CC):
                        nc.sync.dma_start(fch[:, cc, :],
                                          feat_h[t, cc * 128:(cc + 1) * 128, :])
                    nc.vector.tensor_copy(f16v[:], fch[:])
                else:
                    for cc in range(CC):
                        nc.gpsimd.dma_start(f16v[:, cc, :],
                                            feat_h[t, cc * 128:(cc + 1) * 128, :])
                if i == 0:
                    nc.sync.dma_start(esel_sb[:], handles["esel"].ap())
                    nc.sync.dma_start(i16_sb[:], handles["i16"].ap())
                    dummy_sb = sp.tile([1, 1], FP32, tag="dummy")
                    nc.scalar.activation(dummy_sb[:], i16_sb[0:1, 0:1],
                                         AF.Sqrt)
                sq = sp.tile([128, CC, P], F16, tag="sq")
                nc.vector.tensor_mul(sq[:], f16v[:], f16v[:])
                gsc = sp.tile([128, CC, P], F16, tag="gsc")
                nc.vector.tensor_scalar(gsc[:], f16v[:], 0.0, None, OP.is_gt)
                st = (i == 0)
                sx = (i == len(order) - 1)
                lhs = esel_sb[:, T * t:T * (t + 1)]
                for cc in range(CC):
                    for ph in range(PH):
                        sl = slice(ph * 512, (ph + 1) * 512)
                        for j, src in enumerate((f16v, sq, gsc)):
                            nc.tensor.matmul(
                                st_ps[ph][j][32 * j:32 * j + T, :], lhs,
                                src[:, cc, sl],
                                start=st and cc == 0,
                                stop=sx and cc == CC - 1,
                                tile_position=(0, 32 * j))

            for name, t_ in (("wt3", wt3_sb), ("bcol", bcol_sb),
                             ("diagbig", diag_sb), ("tcolu", tcolu_sb)):
                nc.sync.dma_start(t_[:], handles[name].ap())

            for ph in range(PH):
                sl = slice(ph * 512, (ph + 1) * 512)
                nc.scalar.activation(stats_sb[32:32 + T, sl],
                                     st_ps[ph][1][32:32 + T, :], AF.Sqrt)
            for ph in range(PH):
                sl = slice(ph * 512, (ph + 1) * 512)
                nc.vector.tensor_copy(stats_sb[0:T, sl],
                                      st_ps[ph][0][0:T, :])
            for ph in range(PH):
                sl = slice(ph * 512, (ph + 1) * 512)
                nc.scalar.copy(stats_sb[64:64 + T, sl],
                               st_ps[ph][2][64:64 + T, :])

            TRM = {"rs": 0, "sm": 1, "gm": 2}
            for half in range(2):
                for key, stsrc, ibase in (("rs", rs_sb, 32), ("sm", sm_sb, 0)):
                    ident = i16_sb[ibase:ibase + T, :]
                    for pb in range(half * 4, half * 4 + 4):
                        nc.tensor.transpose(
                            trall[:, TRM[key], pb * T:(pb + 1) * T],
                            stsrc[:, pb * 128:(pb + 1) * 128], ident)
                hsl = slice(half * 4, half * 4 + 4)
                hfl = slice(half * 4 * T, (half * 4 + 4) * T)
                nc.vector.reciprocal(rst_sb[:, hsl, :], trall[:, 0, hfl])
                nc.vector.tensor_mul(spt_sb[:, hsl, :], trall[:, 1, hfl],
                                     rst_sb[:, hsl, :])
            ident = i16_sb[64:64 + T, :]
            for pb in range(PB):
                nc.tensor.transpose(trall[:, 2, pb * T:(pb + 1) * T],
                                    gm_sb[:, pb * 128:(pb + 1) * 128], ident)
                if pb % 4 == 3:
                    hfl = slice((pb - 3) * T, (pb + 1) * T)
                    nc.scalar.copy(mpt_sb[:, pb - 3:pb + 1, :],
                                   trall[:, 2, hfl])

            for pb in range(PB):
                nc.tensor.matmul(sc_ps[:], spt_sb[:, pb, :], mpt_sb[:, pb, :],
                                 start=(pb == 0), stop=(pb == PB - 1))
            nc.vector.tensor_sub(scores_sb[:], sc_ps[:], diag_sb[:])

            nc.vector.max(maxv_sb[:], scores_sb[:])
            nc.vector.max_index(pad32_sb[0:T, 4:12], maxv_sb[:], scores_sb[:])
            nc.vector.tensor_tensor(gtu_sb[:], pad32_sb[0:T, 4:4 + K],
                                    tcolu_sb[:], OP.is_gt)
            nc.vector.tensor_sub(pad32_sb[0:T, 0:K], pad32_sb[0:T, 4:4 + K],
                                 gtu_sb[:])
            nc.vector.transpose(zt_sb[:], pad32_sb[:])
            if debug:
                nc.sync.dma_start(handles["scores_dbg"].ap(), scores_sb[:])
                nc.sync.dma_start(handles["idx_dbg"].ap(), zt_sb[0:K, 0:T])

        with tc.tile_pool(name="cps", bufs=4, space="PSUM") as cps, \
             tc.tile_pool(name="cpool", bufs=3) as cp:
            H = T // 2

            def vload(row, lo):
                _, v = nc.values_load_multi_w_load_instructions(
                    zt_sb[row:row + 1, lo:lo + H],
                    engines=bass.OrderedSet([ET.DVE]),
                    min_val=0, max_val=T - 2,
                    skip_runtime_bounds_check=True,
                )
                return list(v)

            v0 = vload(0, 0)
            v1 = vload(1, 0)
            v2 = vload(2, 0)
            loaded = H

            for t in range(T):
                if t >= loaded:
                    v0 += vload(0, loaded)
                    v1 += vload(1, loaded)
                    v2 += vload(2, loaded)
                    loaded += H
                mf16 = cp.tile([128, CC, P], F16, tag="mf16")
                nch = PH if t < 2 else 1
                w = P // nch
                for q in range(nch):
                    sl = slice(q * w, (q + 1) * w)
                    for cc in range(CC):
                        a0 = f16_sb[:, cc, bass.ds(v0[t] * P + q * w, w)]
                        a1 = f16_sb[:, cc, bass.ds(v1[t] * P + q * w, w)]
                        a2 = f16_sb[:, cc, bass.ds(v2[t] * P + q * w, w)]
                        nc.vector.tensor_add(mf16[:, cc, sl], a0, a1)
                        nc.vector.tensor_add(mf16[:, cc, sl], mf16[:, cc, sl],
                                             a2)
                for dc in range(DC):
                    osb = cp.tile([128, P], F16, tag="osb", bufs=4)
                    po = cps.tile([128, P], FP32, tag="po")
                    for ph in range(PH):
                        for cc in range(CC):
                            nc.tensor.matmul(
                                po[:, ph * 512:(ph + 1) * 512],
                                wt3_sb[:, cc, dc * 128:(dc + 1) * 128],
                                mf16[:, cc, ph * 512:(ph + 1) * 512],
                                start=(cc == 0), stop=(cc == CC - 1),
                            )
                        if t < 2:
                            sl = slice(ph * 512, (ph + 1) * 512)
                            nc.scalar.activation(osb[:, sl], po[:, sl],
                                                 AF.Identity,
                                                 bias=bcol_sb[:, dc:dc + 1])
                            nc.sync.dma_start(
                                out_h[t, dc * 128:(dc + 1) * 128, sl],
                                osb[:, sl])
                    if t >= 2:
                        nc.scalar.activation(osb[:], po[:], AF.Identity,
                                             bias=bcol_sb[:, dc:dc + 1])
                        nc.sync.dma_start(out_h[t, dc * 128:(dc + 1) * 128, :],
                                          osb[:])


def build_program(T=16, C=256, P=1024, K=3, debug=False):
    nc = bacc.Bacc("TRN2", target_bir_lowering=False, debug=False,
                   num_devices=N_CORES)
    handles = {}
    handles["features"] = nc.dram_tensor("features", [T, C, P], FP32,
                                         kind="ExternalInput")
    for name, shape, dt in (
        ("wt3", [128, C // 128, C], F16),
        ("bcol", [128, C // 128], FP32),
        ("esel", [128, T * T], F16),
        ("i16", [96, T], FP32),
        ("diagbig", [T, T], FP32),
        ("tcolu", [T, K], U32),
    ):
        handles[name] = nc.dram_tensor(name, shape, dt, kind="ExternalInput")
    handles["out"] = nc.dram_tensor("out", [T, C, P], F16, kind="ExternalOutput")
    if debug:
        handles["scores_dbg"] = nc.dram_tensor("scores_dbg", [T, T], FP32,
                                               kind="ExternalOutput")
        handles["idx_dbg"] = nc.dram_tensor("idx_dbg", [K, T], U32,
                                            kind="ExternalOutput")

    with tile.TileContext(nc) as tc:
        _emit(nc, tc, T, C, P, K, handles, debug)
    nc.compile()
    return nc


def _host_consts(W, b, T, C, K):
    consts = {}
    wt3 = (np.asarray(W, np.float32).T / float(K)).astype(np.float32)
    w4 = wt3.reshape(C // 128, 128, C).transpose(1, 0, 2)
    consts["wt3"] = np.ascontiguousarray(w4.astype(np.float16))
    consts["bcol"] = np.ascontiguousarray(
        np.asarray(b, np.float32).reshape(C // 128, 128).T)
    esel = np.zeros((128, T * T), np.float16)
    for t in range(T):
        esel[:, T * t + t] = 1.0
    consts["esel"] = esel
    i16 = np.zeros((96, T), np.float32)
    for r in (0, 32, 64):
        i16[r:r + T, :] = np.eye(T, dtype=np.float32)
    consts["i16"] = i16
    consts["diagbig"] = (np.eye(T, dtype=np.float32) * BIG).astype(np.float32)
    consts["tcolu"] = np.broadcast_to(
        np.arange(T, dtype=np.uint32).reshape(T, 1), (T, K)).copy()
    return consts


_CACHE = {}


def kernel(features, W, b, top_k):
    features = np.asarray(features, np.float32)
    T, B, C, H, Wd = features.shape
    P = H * Wd
    K = int(top_k)
    assert B == N_CORES and C == 256 and P == 1024 and T == 16 and K == 3

    key = (T, C, P, K)
    if key not in _CACHE:
        _CACHE[key] = build_program(T, C, P, K)
    nc = _CACHE[key]

    consts = _host_consts(W, b, T, C, K)
    feat = features.reshape(T, B, C, P)
    in_maps = [
        {"features": np.ascontiguousarray(feat[:, i]), **consts}
        for i in range(N_CORES)
    ]
    res = None
    last_exc = None
    for _attempt in range(12):
        try:
            res = run_bass_kernel_spmd(nc, in_maps, list(range(N_CORES)))
            break
        except Exception as e:  # transient device/tunnel failures
            last_exc = e
            import time as _time
            _time.sleep(3.0)
    if res is None:
        raise last_exc
    out = np.stack([res.results[i]["out"] for i in range(N_CORES)],
                   axis=1).astype(np.float32)
    return np.ascontiguousarray(out.reshape(T, B, C, H, Wd))


# revision 19
# speedup vs baseline: 1.0207x; 1.0207x over previous
"""Trainium2 Bass kernel for CrossFrameSimilarityRefiner.

Computation (per batch element b, fully batch-parallel -> B=8 sharded over 8 cores):
  f = features[:, b]                      # [T, C, P]  T=16, C=256, P=1024
  ss[t,p] = sum_c f^2 ; sm[t,p] = sum_c f ; gm[t,p] = sum_c (f>0)
  S[t,p]  = sm / sqrt(ss)
  scores[t,s] = sum_p S[t,p] * gm[s,p]    # same per-row ranking as reference
  mask diag, top-3 indices, compressed c* = s* - (s* > t)  (faithful bug)
  out[t] = (W/3) @ (f[c*0]+f[c*1]+f[c*2]) + b

Pipeline: startup ~7us -> HBM-bound input stream ~46us (frames 13,14 on the
sync ring first for warmup, 0..12 via software-DGE cast-DMA, frame 15 on the
sync ring LAST so the stats-psum stopper rides a fast completion semaphore
instead of the laggy DGE one) -> bridge (stats evac, PE transposes, score
matmuls, top-k, index math) -> phase C (per t: two DVE gather-adds, 8 PE
matmuls, ACT psum-evac with fused bias, fp16 out-DMA; host casts to fp32).
"""

import numpy as np

import concourse.bacc as bacc
import concourse.bass as bass
import concourse.tile as tile
from concourse import mybir
from concourse.bass_utils import run_bass_kernel_spmd

FP32 = mybir.dt.float32
F16 = mybir.dt.float16
I32 = mybir.dt.int32
U32 = mybir.dt.uint32
AF = mybir.ActivationFunctionType
OP = mybir.AluOpType
ET = mybir.EngineType

N_CORES = 8
BIG = 1.0e30


def _emit(nc, tc, T, C, P, K, handles, debug):
    feat_h = handles["features"]
    out_h = handles["out"]
    CC = C // 128
    PH = P // 512
    PB = P // 128
    DC = C // 128

    with tc.tile_pool(name="persist", bufs=1) as pp:
        wt3_sb = pp.tile([128, CC, C], F16, tag="wt3")
        bcol_sb = pp.tile([128, DC], FP32, tag="bcol")
        esel_sb = pp.tile([128, T * T], F16, tag="esel")
        i16_sb = pp.tile([96, T], FP32, tag="i16")
        diag_sb = pp.tile([T, T], FP32, tag="diag")
        tcolu_sb = pp.tile([T, K], U32, tag="tcolu")

        f16_sb = pp.tile([128, CC, T * P], F16, tag="f16")
        stats_sb = pp.tile([96, P], FP32, tag="stats")
        sm_sb = stats_sb[0:T, :]
        rs_sb = stats_sb[32:32 + T, :]
        gm_sb = stats_sb[64:64 + T, :]
        rst_sb = pp.tile([128, PB, T], FP32, tag="rsT")
        spt_sb = pp.tile([128, PB, T], FP32, tag="SpT")
        mpt_sb = pp.tile([128, PB, T], FP32, tag="MpT")
        scores_sb = pp.tile([T, T], FP32, tag="scores")
        maxv_sb = pp.tile([T, 8], FP32, tag="maxv")
        pad32_sb = pp.tile([32, 32], U32, tag="pad32")
        zt_sb = pp.tile([32, 32], U32, tag="zt")
        gtu_sb = pp.tile([T, K], U32, tag="gtu")

        with tc.tile_pool(name="statsps", bufs=1, space="PSUM") as sps, \
             tc.tile_pool(name="bps", bufs=1, space="PSUM") as bps, \
             tc.tile_pool(name="stream", bufs=4) as sp:
            st_ps = [[sps.tile([96, 512], FP32, tag=f"stp{ph}_{j}",
                               name=f"stp{ph}_{j}") for j in range(3)]
                     for ph in range(PH)]
            trall = bps.tile([128, 3, PB * T], FP32, tag="trall", name="trall")
            sc_ps = bps.tile([T, T], FP32, tag="scps", name="scps")

            nc.gpsimd.memset(pad32_sb[:], 0)

            early = [T - 3, T - 2, T - 1]
            order = [T - 3, T - 2] + list(range(T - 3)) + [T - 1]
            for i, t in enumerate(order):
                f16v = f16_sb[:, :, t * P:(t + 1) * P]
                if t in early:
                    fch = sp.tile([128, CC, P], FP32, tag="fch")
                    for cc in range(CC):
                        nc.sync.dma_start(fch[:, cc, :],
                                          feat_h[t, cc * 128:(cc + 1) * 128, :])
                    nc.vector.tensor_copy(f16v[:], fch[:])
                else:
                    for cc in range(CC):
                        nc.gpsimd.dma_start(f16v[:, cc, :],
                                            feat_h[t, cc * 128:(cc + 1) * 128, :])
                if i == 0:
                    nc.sync.dma_start(esel_sb[:], handles["esel"].ap())
                    nc.sync.dma_start(i16_sb[:], handles["i16"].ap())
                    dummy_sb = sp.tile([1, 1], FP32, tag="dummy")
                    nc.scalar.activation(dummy_sb[:], i16_sb[0:1, 0:1],
                                         AF.Sqrt)
                sq = sp.tile([128, CC, P], F16, tag="sq")
                nc.vector.tensor_mul(sq[:], f16v[:], f16v[:])
                gsc = sp.tile([128, CC, P], F16, tag="gsc")
                nc.vector.tensor_scalar(gsc[:], f16v[:], 0.0, None, OP.is_gt)
                st = (i == 0)
                sx = (i == len(order) - 1)
                lhs = esel_sb[:, T * t:T * (t + 1)]
                for cc in range(CC):
                    for ph in range(PH):
                        sl = slice(ph * 512, (ph + 1) * 512)
                        for j, src in enumerate((f16v, sq, gsc)):
                            nc.tensor.matmul(
                                st_ps[ph][j][32 * j:32 * j + T, :], lhs,
                                src[:, cc, sl],
                                start=st and cc == 0,
                                stop=sx and cc == CC - 1,
                                tile_position=(0, 32 * j))

            for name, t_ in (("wt3", wt3_sb), ("bcol", bcol_sb),
                             ("diagbig", diag_sb), ("tcolu", tcolu_sb)):
                nc.sync.dma_start(t_[:], handles[name].ap())

            for ph in range(PH):
                sl = slice(ph * 512, (ph + 1) * 512)
                nc.scalar.activation(stats_sb[32:32 + T, sl],
                                     st_ps[ph][1][32:32 + T, :], AF.Sqrt)
            for ph in range(PH):
                sl = slice(ph * 512, (ph + 1) * 512)
                nc.vector.tensor_copy(stats_sb[0:T, sl],
                                      st_ps[ph][0][0:T, :])
            for ph in range(PH):
                sl = slice(ph * 512, (ph + 1) * 512)
                nc.scalar.copy(stats_sb[64:64 + T, sl],
                               st_ps[ph][2][64:64 + T, :])

            TRM = {"rs": 0, "sm": 1, "gm": 2}
            for half in range(2):
                for key, stsrc, ibase in (("rs", rs_sb, 32), ("sm", sm_sb, 0)):
                    ident = i16_sb[ibase:ibase + T, :]
                    for pb in range(half * 4, half * 4 + 4):
                        nc.tensor.transpose(
                            trall[:, TRM[key], pb * T:(pb + 1) * T],
                            stsrc[:, pb * 128:(pb + 1) * 128], ident)
                hsl = slice(half * 4, half * 4 + 4)
                hfl = slice(half * 4 * T, (half * 4 + 4) * T)
                nc.vector.reciprocal(rst_sb[:, hsl, :], trall[:, 0, hfl])
                nc.vector.tensor_mul(spt_sb[:, hsl, :], trall[:, 1, hfl],
                                     rst_sb[:, hsl, :])
            ident = i16_sb[64:64 + T, :]
            for pb in range(PB):
                nc.tensor.transpose(trall[:, 2, pb * T:(pb + 1) * T],
                                    gm_sb[:, pb * 128:(pb + 1) * 128], ident)
                if pb % 4 == 3:
                    hfl = slice((pb - 3) * T, (pb + 1) * T)
                    nc.scalar.copy(mpt_sb[:, pb - 3:pb + 1, :],
                                   trall[:, 2, hfl])

            for pb in range(PB):
                nc.tensor.matmul(sc_ps[:], spt_sb[:, pb, :], mpt_sb[:, pb, :],
                                 start=(pb == 0), stop=(pb == PB - 1))
            nc.vector.tensor_sub(scores_sb[:], sc_ps[:], diag_sb[:])

            nc.vector.max(maxv_sb[:], scores_sb[:])
            nc.vector.max_index(pad32_sb[0:T, 4:12], maxv_sb[:], scores_sb[:])
            nc.vector.tensor_tensor(gtu_sb[:], pad32_sb[0:T, 4:4 + K],
                                    tcolu_sb[:], OP.is_gt)
            nc.vector.tensor_sub(pad32_sb[0:T, 0:K], pad32_sb[0:T, 4:4 + K],
                                 gtu_sb[:])
            nc.vector.transpose(zt_sb[:], pad32_sb[:])
            if debug:
                nc.sync.dma_start(handles["scores_dbg"].ap(), scores_sb[:])
                nc.sync.dma_start(handles["idx_dbg"].ap(), zt_sb[0:K, 0:T])

        with tc.tile_pool(name="cps", bufs=4, space="PSUM") as cps, \
             tc.tile_pool(name="cpool", bufs=3) as cp:
            H = T // 2

            def vload(row, lo):
                _, v = nc.values_load_multi_w_load_instructions(
                    zt_sb[row:row + 1, lo:lo + H],
                    engines=bass.OrderedSet([ET.DVE]),
                    min_val=0, max_val=T - 2,
                    skip_runtime_bounds_check=True,
                )
                return list(v)

            v0 = vload(0, 0)
            v1 = vload(1, 0)
            v2 = vload(2, 0)
            loaded = H

            for t in range(T):
                if t >= loaded:
                    v0 += vload(0, loaded)
                    v1 += vload(1, loaded)
                    v2 += vload(2, loaded)
                    loaded += H
                mf16 = cp.tile([128, CC, P], F16, tag="mf16")
                nch = PH if t < 2 else 1
                w = P // nch
                for q in range(nch):
                    sl = slice(q * w, (q + 1) * w)
                    for cc in range(CC):
                        a0 = f16_sb[:, cc, bass.ds(v0[t] * P + q * w, w)]
                        a1 = f16_sb[:, cc, bass.ds(v1[t] * P + q * w, w)]
                        a2 = f16_sb[:, cc, bass.ds(v2[t] * P + q * w, w)]
                        nc.vector.tensor_add(mf16[:, cc, sl], a0, a1)
                        nc.vector.tensor_add(mf16[:, cc, sl], mf16[:, cc, sl],
                                             a2)
                for dc in range(DC):
                    osb = cp.tile([128, P], F16, tag="osb", bufs=4)
                    po = cps.tile([128, P], FP32, tag="po")
                    for ph in range(PH):
                        for cc in range(CC):
                            nc.tensor.matmul(
                                po[:, ph * 512:(ph + 1) * 512],
                                wt3_sb[:, cc, dc * 128:(dc + 1) * 128],
                                mf16[:, cc, ph * 512:(ph + 1) * 512],
                                start=(cc == 0), stop=(cc == CC - 1),
                            )
                        if t < 2:
                            sl = slice(ph * 512, (ph + 1) * 512)
                            nc.scalar.activation(osb[:, sl], po[:, sl],
                                                 AF.Identity,
                                                 bias=bcol_sb[:, dc:dc + 1])
                            nc.sync.dma_start(
                                out_h[t, dc * 128:(dc + 1) * 128, sl],
                                osb[:, sl])
                    if t >= 2:
                        nc.scalar.activation(osb[:], po[:], AF.Identity,
                                             bias=bcol_sb[:, dc:dc + 1])
                        nc.sync.dma_start(out_h[t, dc * 128:(dc + 1) * 128, :],
                                          osb[:])


def build_program(T=16, C=256, P=1024, K=3, debug=False):
    nc = bacc.Bacc("TRN2", target_bir_lowering=False, debug=False,
                   num_devices=N_CORES)
    handles = {}
    handles["features"] = nc.dram_tensor("features", [T, C, P], FP32,
                                         kind="ExternalInput")
    for name, shape, dt in (
        ("wt3", [128, C // 128, C], F16),
        ("bcol", [128, C // 128], FP32),
        ("esel", [128, T * T], F16),
        ("i16", [96, T], FP32),
        ("diagbig", [T, T], FP32),
        ("tcolu", [T, K], U32),
    ):
        handles[name] = nc.dram_tensor(name, shape, dt, kind="ExternalInput")
    handles["out"] = nc.dram_tensor("out", [T, C, P], F16, kind="ExternalOutput")
    if debug:
        handles["scores_dbg"] = nc.dram_tensor("scores_dbg", [T, T], FP32,
                                               kind="ExternalOutput")
        handles["idx_dbg"] = nc.dram_tensor("idx_dbg", [K, T], U32,
                                            kind="ExternalOutput")

    with tile.TileContext(nc) as tc:
        _emit(nc, tc, T, C, P, K, handles, debug)
    nc.compile()
    return nc


def _host_consts(W, b, T, C, K):
    consts = {}
    wt3 = (np.asarray(W, np.float32).T / float(K)).astype(np.float32)
    w4 = wt3.reshape(C // 128, 128, C).transpose(1, 0, 2)
    consts["wt3"] = np.ascontiguousarray(w4.astype(np.float16))
    consts["bcol"] = np.ascontiguousarray(
        np.asarray(b, np.float32).reshape(C // 128, 128).T)
    esel = np.zeros((128, T * T), np.float16)
    for t in range(T):
        esel[:, T * t + t] = 1.0
    consts["esel"] = esel
    i16 = np.zeros((96, T), np.float32)
    for r in (0, 32, 64):
        i16[r:r + T, :] = np.eye(T, dtype=np.float32)
    consts["i16"] = i16
    consts["diagbig"] = (np.eye(T, dtype=np.float32) * BIG).astype(np.float32)
    consts["tcolu"] = np.broadcast_to(
        np.arange(T, dtype=np.uint32).reshape(T, 1), (T, K)).copy()
    return consts


_CACHE = {}


def kernel(features, W, b, top_k):
    features = np.asarray(features, np.float32)
    T, B, C, H, Wd = features.shape
    P = H * Wd
    K = int(top_k)
    assert B == N_CORES and C == 256 and P == 1024 and T == 16 and K == 3

    key = (T, C, P, K)
    if key not in _CACHE:
        _CACHE[key] = build_program(T, C, P, K)
    nc = _CACHE[key]

    consts = _host_consts(W, b, T, C, K)
    feat = features.reshape(T, B, C, P)
    in_maps = [
        {"features": np.ascontiguousarray(feat[:, i]), **consts}
        for i in range(N_CORES)
    ]
    res = None
    last_exc = None
    # transient device/tunnel failures come in minutes-long windows;
    # progressive backoff waits them out
    _sleeps = (3, 5, 8, 12, 18, 25, 30, 30, 30, 30)
    for _attempt, _s in enumerate(_sleeps + (0,)):
        try:
            res = run_bass_kernel_spmd(nc, in_maps, list(range(N_CORES)))
            break
        except Exception as e:
            last_exc = e
            import time as _time
            _time.sleep(_s)
    if res is None:
        raise last_exc
    out = np.stack([res.results[i]["out"] for i in range(N_CORES)],
                   axis=1).astype(np.float32)
    return np.ascontiguousarray(out.reshape(T, B, C, H, Wd))
